# revision 10
# baseline (speedup 1.0000x reference)
"""ClusterLayer (vq_codebook) Trainium2 kernel.

Math (labels are one-hot over K clusters, i(n) = argmax labels[n]):
  nume[n]  = ||x_n - c_{i(n)}||^2 = ||x_n||^2 - 2 x_n.c_{i(n)} + ||c_{i(n)}||^2
  denom[k] = sum_j ||c_k - c_j||^2 + ALPHA = K*cn[k] + S - 2*Grs[k] + ALPHA
  loss[n]  = nume[n] / denom[i(n)]
  new_cluster[k] = cluster[k]*(1 + b[k]*count[k]) - b[k]*Sx[k]   with
  b[k] = GAMMA*cw[k]/denom[k], Sx = L^T @ X (segment sums), count = colsum(L)

Sharding: data-parallel over N across 8 cores. Rows are assigned to
row-tiles by n % 16 (tile j holds rows {p*16+j}) so the labels load is one
contiguous DMA; every per-row quantity is permutation-invariant.

Schedule: per tile the Sx matmuls are emitted before the ci/loss work, so
the Sx accumulator finishes as soon as the last X tile lands; the single
AllReduce of [Sx | count] launches right then, overlapping the remaining
per-row loss work, and only its latency tail is exposed.

Engine split: PE does Sx (fp32r) + ci = L@C (bf16; one-hot labels exact,
cluster rounding washes out in nume ~ 2048); DVE does the fp32 x.ci row
dots (fp32 tensor_tensor is 1x on DVE - hard floor); ACT does ||x||^2
squares and all PSUM->SBUF copies; per-row scalar fixups are batched in
groups of 4 row tiles.
"""

import sys

if "/opt/trn_rl_repo" not in sys.path:
    sys.path.insert(0, "/opt/trn_rl_repo")

import numpy as np

import concourse.bass as bass
import concourse.bacc as bacc
import concourse.tile as tile
import concourse.mybir as mybir
from concourse import bass_utils

N, K, F = 16384, 64, 2048
ALPHA, GAMMA = 1.0, 1.0
N_CORES = 8
NLOC = N // N_CORES          # 2048 rows per core
T = NLOC // 128              # 16 row tiles per core
FCH = F // 128               # 16 column chunks of cluster
GRP = 4                      # row tiles per small-op batch
f32 = mybir.dt.float32
f32r = mybir.dt.float32r
bf16 = mybir.dt.bfloat16
AF = mybir.ActivationFunctionType
ALU = mybir.AluOpType
AX = mybir.AxisListType

_CACHED_NC = None


def _build():
    nc = bacc.Bacc("TRN2", target_bir_lowering=False, debug=False,
                   num_devices=N_CORES)
    Xr = nc.dram_tensor("features_r", [NLOC, F], f32r, kind="ExternalInput").ap()
    Lf = nc.dram_tensor("labels_f", [NLOC, K], f32, kind="ExternalInput").ap()
    Cf = nc.dram_tensor("cluster_f", [K, F], f32, kind="ExternalInput").ap()
    cw = nc.dram_tensor("class_weight", [K, 1], f32, kind="ExternalInput").ap()
    ident = nc.dram_tensor("identity", [128, 128], f32, kind="ExternalInput").ap()
    loss_out = nc.dram_tensor("loss_out", [NLOC, 1], f32, kind="ExternalOutput").ap()
    clus_out = nc.dram_tensor("cluster_out", [K, F], f32, kind="ExternalOutput").ap()

    # row-tile view: tile j holds rows {p*16 + j, p in 0..127}
    Xv = Xr.rearrange("(p t) f -> p t f", t=T)
    Lv = Lf.rearrange("(p t) k -> p t k", t=T)

    with tile.TileContext(nc) as tc:
        with tc.tile_pool(name="const", bufs=1) as cpool, \
             tc.tile_pool(name="xin", bufs=6) as xpool, \
             tc.tile_pool(name="junk", bufs=1) as jpool, \
             tc.tile_pool(name="small", bufs=1) as spool, \
             tc.tile_pool(name="psSx", bufs=1, space="PSUM") as psA, \
             tc.tile_pool(name="dram", bufs=1, space="DRAM") as dpool:

            # ---------------- constant loads ----------------
            C_f = cpool.tile([K, F], f32)
            nc.sync.dma_start(C_f[:], Cf[:])
            id_t = cpool.tile([128, 128], f32)
            nc.sync.dma_start(id_t[:], ident[:])
            L_f = cpool.tile([128, T * K], f32)  # tile j at cols [j*K,(j+1)*K)
            nc.sync.dma_start(L_f[:].rearrange("p (t k) -> p t k", t=T), Lv)
            L_r = cpool.tile([128, T * K], f32r)
            nc.sync.dma_start(L_r[:], L_f[:].bitcast(f32r))  # rebless f32->f32r
            C16 = cpool.tile([K, F], bf16)
            nc.scalar.copy(C16[:], C_f[:])
            cw_t = spool.tile([K, 1], f32)
            nc.sync.dma_start(cw_t[:], cw[:])

            # ---------------- transposes via PE ----------------
            LT_f = cpool.tile([K, NLOC], f32)    # labels^T (f32)
            LT16 = cpool.tile([K, NLOC], bf16)   # labels^T (bf16, ci matmul)
            CT = cpool.tile([128, FCH * K], f32)  # cluster^T chunks [128_f, K]
            with tc.tile_pool(name="psT", bufs=2, space="PSUM") as psT:
                for t in range(T):
                    sl = slice(t * 128, (t + 1) * 128)
                    ps = psT.tile([K, 128], f32, tag="tr")
                    nc.tensor.transpose(ps[:], L_f[:, t * K:(t + 1) * K], id_t[:])
                    nc.scalar.copy(LT_f[:, sl], ps[:])
                    nc.scalar.copy(LT16[:, sl], ps[:])
                for fc in range(FCH):
                    ps2 = psT.tile([128, K], f32, tag="tr2")
                    nc.tensor.transpose(ps2[:], C_f[:, fc * 128:(fc + 1) * 128],
                                        id_t[0:K, 0:K])
                    nc.scalar.copy(CT[:, fc * K:(fc + 1) * K], ps2[:])

            # ---------------- cluster geometry (prelude) ----------------
            with tc.tile_pool(name="psM", bufs=1, space="PSUM") as psM:
                junkC = jpool.tile([K, F], f32, tag="junk2")
                cn = spool.tile([K, 1], f32)
                nc.scalar.activation(junkC[:], C_f[:], AF.Square,
                                     accum_out=cn[:])
                G_ps = psM.tile([K, K], f32, tag="m")
                for fc in range(FCH):
                    ct = CT[:, fc * K:(fc + 1) * K]
                    nc.tensor.matmul(G_ps[:], ct, ct,
                                     start=(fc == 0), stop=(fc == FCH - 1))
                Grs = spool.tile([K, 1], f32)
                nc.vector.tensor_reduce(Grs[:], G_ps[:], AX.X, ALU.add)
                ones64 = spool.tile([K, 1], f32)
                nc.vector.memset(ones64[:], 1.0)
                ones1 = spool.tile([1, K], f32)
                nc.vector.memset(ones1[:], 1.0)
                S_ps = psM.tile([1, 1], f32, tag="m")
                nc.tensor.matmul(S_ps[:], cn[:], ones64[:], start=True, stop=True)
                S_sb = spool.tile([1, 1], f32)
                nc.vector.tensor_copy(S_sb[:], S_ps[:])
                Sbc_ps = psM.tile([K, 1], f32, tag="m")
                nc.tensor.matmul(Sbc_ps[:], ones1[:], S_sb[:], start=True,
                                 stop=True)

                d1 = spool.tile([K, 1], f32)
                nc.vector.tensor_scalar(d1[:], cn[:], float(K), ALPHA,
                                        ALU.mult, ALU.add)
                d2 = spool.tile([K, 1], f32)
                nc.vector.scalar_tensor_tensor(d2[:], Grs[:], -2.0, d1[:],
                                               ALU.mult, ALU.add)
                denom = spool.tile([K, 1], f32)
                nc.vector.tensor_tensor(denom[:], d2[:], Sbc_ps[:], ALU.add)
                invd = spool.tile([K, 1], f32)
                nc.vector.reciprocal(invd[:], denom[:])
                u = spool.tile([K, 1], f32)      # cn/denom
                nc.vector.tensor_tensor(u[:], cn[:], invd[:], ALU.mult)
                bcoef = spool.tile([K, 1], f32)  # GAMMA*cw/denom
                nc.vector.tensor_tensor(bcoef[:], cw_t[:], invd[:], ALU.mult)
                # per-row gathers: ivr_all[:, t] = invd[i(n)], [:, T+t] = u[i(n)]
                iv_ps = psM.tile([128, 2 * T], f32, tag="m")
                for t in range(T):
                    lt = LT_f[:, t * 128:(t + 1) * 128]
                    nc.tensor.matmul(iv_ps[:, t:t + 1], lt, invd[:],
                                     start=True, stop=True)
                    nc.tensor.matmul(iv_ps[:, T + t:T + t + 1], lt, u[:],
                                     start=True, stop=True)
                ivr_all = cpool.tile([128, 2 * T], f32)
                nc.scalar.copy(ivr_all[:], iv_ps[:])
                count = spool.tile([K, 1], f32)
                nc.vector.tensor_reduce(count[:], LT_f[:], AX.X, ALU.add)

            # ---------------- main loop over row tiles ----------------
            Sx_ps = psA.tile([K, F], f32)  # 4 banks, persistent accumulator
            loss_strip = cpool.tile([128, T], f32, tag="strip")
            Sxs = cpool.tile([K, F], f32, tag="sxs")
            bounce_in = dpool.tile([K, F + 1], f32)
            bounce_out = dpool.tile([K, F + 1], f32, addr_space="Shared")
            rg = [list(range(N_CORES))]
            NG = T // GRP
            with tc.tile_pool(name="psCI", bufs=2, space="PSUM") as psCI:
                for g in range(NG):
                    xsq_g = spool.tile([128, GRP], f32, tag=f"xsq{g % 2}")
                    th0_g = spool.tile([128, GRP], f32, tag=f"th0{g % 2}")
                    th1_g = spool.tile([128, GRP], f32, tag=f"th1{g % 2}")
                    for i in range(GRP):
                        t = g * GRP + i
                        X_t = xpool.tile([128, F], f32r, tag="x")
                        nc.sync.dma_start(X_t[:], Xv[:, t, :])
                        # Sx first: frees the AllReduce as soon as X_15 lands
                        for j in range(4):
                            nc.tensor.matmul(Sx_ps[:, j * 512:(j + 1) * 512],
                                             L_r[:, t * K:(t + 1) * K],
                                             X_t[:, j * 512:(j + 1) * 512],
                                             start=(t == 0), stop=(t == T - 1))
                        if t == T - 1:
                            # scaled partial out + AllReduce, overlapping the
                            # rest of the loss pipeline
                            nc.scalar.mul(Sxs[:], Sx_ps[:], bcoef[:])
                            nc.sync.dma_start(bounce_in[:, 0:F], Sxs[:])
                            nc.sync.dma_start(bounce_in[:, F:F + 1], count[:])
                            nc.gpsimd.collective_compute(
                                "AllReduce", ALU.add, replica_groups=rg,
                                ins=[bounce_in.opt()], outs=[bounce_out.opt()])
                        lt16 = LT16[:, t * 128:(t + 1) * 128]
                        for h in range(2):
                            ci_ps = psCI.tile([128, F // 2], f32, tag="ci")
                            for j in range(2):
                                cs = slice(h * 1024 + j * 512,
                                           h * 1024 + (j + 1) * 512)
                                nc.tensor.matmul(ci_ps[:, j * 512:(j + 1) * 512],
                                                 lt16, C16[:, cs],
                                                 start=True, stop=True)
                            junk1 = jpool.tile([128, F // 2], f32, tag="junk1")
                            # fused x*ci row-dot (tensor_tensor_reduce is
                            # broken on this runtime; scalar_tensor_tensor
                            # with accum_out is the working equivalent)
                            nc.vector.scalar_tensor_tensor(
                                junk1[:],
                                X_t[:, h * 1024:(h + 1) * 1024].bitcast(f32),
                                1.0, ci_ps[:],
                                ALU.mult, ALU.mult,
                                accum_out=(th0_g[:, i:i + 1] if h == 0
                                           else th1_g[:, i:i + 1]))
                        junk2 = jpool.tile([128, F], f32, tag="junk2")
                        nc.scalar.activation(junk2[:], X_t[:].bitcast(f32),
                                             AF.Square,
                                             accum_out=xsq_g[:, i:i + 1])
                    # batched per-row fixup for the group: loss =
                    # (xsq - 2*(th0+th1))*invd_row + u_row
                    trow_g = spool.tile([128, GRP], f32, tag=f"trow{g % 2}")
                    nc.vector.tensor_tensor(trow_g[:], th0_g[:], th1_g[:],
                                            ALU.add)
                    tmp_g = spool.tile([128, GRP], f32, tag=f"tmp{g % 2}")
                    nc.vector.scalar_tensor_tensor(tmp_g[:], trow_g[:], -2.0,
                                                   xsq_g[:], ALU.mult, ALU.add)
                    nc.vector.tensor_tensor(
                        tmp_g[:], tmp_g[:],
                        ivr_all[:, g * GRP:(g + 1) * GRP], ALU.mult)
                    nc.vector.tensor_tensor(
                        loss_strip[:, g * GRP:(g + 1) * GRP], tmp_g[:],
                        ivr_all[:, T + g * GRP:T + (g + 1) * GRP], ALU.add)
                nc.sync.dma_start(
                    loss_out.rearrange("(p t) o -> p t o", t=T),
                    loss_strip[:].rearrange("p (t o) -> p t o", o=1))

            # ---------------- cluster update ----------------
            red = cpool.tile([K, F + 1], f32, tag="red")
            nc.sync.dma_start(red[:], bounce_out[:])
            q = spool.tile([K, 1], f32)
            nc.vector.tensor_scalar(q[:], red[:, F:F + 1], bcoef[:], 1.0,
                                    ALU.mult, ALU.add)
            newC = cpool.tile([K, F], f32, tag="newC")
            nc.vector.scalar_tensor_tensor(newC[:], C_f[:], q[:],
                                           red[:, 0:F], ALU.mult,
                                           ALU.subtract)
            nc.sync.dma_start(clus_out[:], newC[:])

    nc.compile()
    return nc


def _get_nc():
    global _CACHED_NC
    if _CACHED_NC is None:
        _CACHED_NC = _build()
    return _CACHED_NC


def _in_maps(features, labels, cluster, class_weight):
    features = np.ascontiguousarray(features, dtype=np.float32)
    labels = np.ascontiguousarray(labels, dtype=np.float32)
    cluster = np.ascontiguousarray(cluster, dtype=np.float32)
    cw = np.ascontiguousarray(class_weight, dtype=np.float32).reshape(K, 1)
    eye = np.eye(128, dtype=np.float32)
    maps = []
    for c in range(N_CORES):
        sl = slice(c * NLOC, (c + 1) * NLOC)
        maps.append({
            "features_r": np.ascontiguousarray(features[sl]),
            "labels_f": np.ascontiguousarray(labels[sl]),
            "cluster_f": cluster,
            "class_weight": cw,
            "identity": eye,
        })
    return maps


def run(features, labels, cluster, class_weight, **run_kwargs):
    nc = _get_nc()
    maps = _in_maps(features, labels, cluster, class_weight)
    res = bass_utils.run_bass_kernel_spmd(
        nc, maps, core_ids=list(range(N_CORES)), **run_kwargs)
    loss = np.concatenate(
        [res.results[c]["loss_out"] for c in range(N_CORES)], axis=0)
    new_cluster = res.results[0]["cluster_out"]
    return (loss, new_cluster), res


def kernel(features, labels, cluster, class_weight):
    (loss, new_cluster), _ = run(features, labels, cluster, class_weight)
    return loss, new_cluster


# revision 11
# speedup vs baseline: 1.0599x; 1.0599x over previous
"""ClusterLayer (vq_codebook) Trainium2 kernel.

Math (labels are one-hot over K clusters, i(n) = argmax labels[n]):
  nume[n]  = ||x_n - c_{i(n)}||^2 = ||x_n||^2 - 2 x_n.c_{i(n)} + ||c_{i(n)}||^2
  denom[k] = sum_j ||c_k - c_j||^2 + ALPHA = K*cn[k] + S - 2*Grs[k] + ALPHA
  loss[n]  = nume[n] / denom[i(n)]
  new_cluster[k] = cluster[k]*(1 + b[k]*count[k]) - b[k]*Sx[k]   with
  b[k] = GAMMA*cw[k]/denom[k], Sx = L^T @ X (segment sums), count = colsum(L)

Sharding: data-parallel over N across 8 cores. Rows are assigned to
row-tiles by n % 16 (tile j holds rows {p*16+j}) so the labels load is one
contiguous DMA; every per-row quantity is permutation-invariant.

Schedule: per tile the Sx matmuls are emitted before the ci/loss work, so
the Sx accumulator finishes as soon as the last X tile lands; the single
AllReduce of [Sx | count] launches right then, overlapping the remaining
per-row loss work, and only its latency tail is exposed.

Engine split: PE does Sx (fp32r) + ci = L@C (bf16; one-hot labels exact,
cluster rounding washes out in nume ~ 2048); DVE does the fp32 x.ci row
dots (fp32 tensor_tensor is 1x on DVE - hard floor); ACT does ||x||^2
squares and all PSUM->SBUF copies; per-row scalar fixups are batched in
groups of 4 row tiles.
"""

import sys

if "/opt/trn_rl_repo" not in sys.path:
    sys.path.insert(0, "/opt/trn_rl_repo")

import numpy as np

import concourse.bass as bass
import concourse.bacc as bacc
import concourse.tile as tile
import concourse.mybir as mybir
from concourse import bass_utils

N, K, F = 16384, 64, 2048
ALPHA, GAMMA = 1.0, 1.0
N_CORES = 8
NLOC = N // N_CORES          # 2048 rows per core
T = NLOC // 128              # 16 row tiles per core
FCH = F // 128               # 16 column chunks of cluster
GRP = 4                      # row tiles per small-op batch
f32 = mybir.dt.float32
f32r = mybir.dt.float32r
bf16 = mybir.dt.bfloat16
AF = mybir.ActivationFunctionType
ALU = mybir.AluOpType
AX = mybir.AxisListType

_CACHED_NC = None


def _build():
    nc = bacc.Bacc("TRN2", target_bir_lowering=False, debug=False,
                   num_devices=N_CORES)
    Xr = nc.dram_tensor("features_r", [NLOC, F], f32r, kind="ExternalInput").ap()
    Lf = nc.dram_tensor("labels_f", [NLOC, K], f32, kind="ExternalInput").ap()
    Cf = nc.dram_tensor("cluster_f", [K, F], f32, kind="ExternalInput").ap()
    cw = nc.dram_tensor("class_weight", [K, 1], f32, kind="ExternalInput").ap()
    ident = nc.dram_tensor("identity", [128, 128], f32, kind="ExternalInput").ap()
    loss_out = nc.dram_tensor("loss_out", [NLOC, 1], f32, kind="ExternalOutput").ap()
    clus_out = nc.dram_tensor("cluster_out", [K, F], f32, kind="ExternalOutput").ap()

    # row-tile view: tile j holds rows {p*16 + j, p in 0..127}
    Xv = Xr.rearrange("(p t) f -> p t f", t=T)
    Lv = Lf.rearrange("(p t) k -> p t k", t=T)

    with tile.TileContext(nc) as tc:
        with tc.tile_pool(name="const", bufs=1) as cpool, \
             tc.tile_pool(name="xin", bufs=6) as xpool, \
             tc.tile_pool(name="junk", bufs=1) as jpool, \
             tc.tile_pool(name="small", bufs=1) as spool, \
             tc.tile_pool(name="psSx", bufs=1, space="PSUM") as psA, \
             tc.tile_pool(name="dram", bufs=1, space="DRAM") as dpool:

            # ---------------- constant loads ----------------
            C_f = cpool.tile([K, F], f32)
            nc.sync.dma_start(C_f[:], Cf[:])
            id_t = cpool.tile([128, 128], f32)
            nc.sync.dma_start(id_t[:], ident[:])
            L_f = cpool.tile([128, T * K], f32)  # tile j at cols [j*K,(j+1)*K)
            nc.sync.dma_start(L_f[:].rearrange("p (t k) -> p t k", t=T), Lv)
            L_r = cpool.tile([128, T * K], f32r)
            nc.sync.dma_start(L_r[:], L_f[:].bitcast(f32r))  # rebless f32->f32r
            C16 = cpool.tile([K, F], bf16)
            nc.scalar.copy(C16[:], C_f[:])
            cw_t = spool.tile([K, 1], f32)
            nc.sync.dma_start(cw_t[:], cw[:])

            # ---------------- transposes via PE ----------------
            LT_f = cpool.tile([K, NLOC], f32)    # labels^T (f32)
            LT16 = cpool.tile([K, NLOC], bf16)   # labels^T (bf16, ci matmul)
            CT = cpool.tile([128, FCH * K], f32)  # cluster^T chunks [128_f, K]
            with tc.tile_pool(name="psT", bufs=2, space="PSUM") as psT:
                for t in range(T):
                    sl = slice(t * 128, (t + 1) * 128)
                    ps = psT.tile([K, 128], f32, tag="tr")
                    nc.tensor.transpose(ps[:], L_f[:, t * K:(t + 1) * K], id_t[:])
                    nc.scalar.copy(LT_f[:, sl], ps[:])
                    nc.scalar.copy(LT16[:, sl], ps[:])
                for fc in range(FCH):
                    ps2 = psT.tile([128, K], f32, tag="tr2")
                    nc.tensor.transpose(ps2[:], C_f[:, fc * 128:(fc + 1) * 128],
                                        id_t[0:K, 0:K])
                    nc.scalar.copy(CT[:, fc * K:(fc + 1) * K], ps2[:])

            # ---------------- cluster geometry (prelude) ----------------
            with tc.tile_pool(name="psM", bufs=1, space="PSUM") as psM:
                junkC = jpool.tile([K, F], f32, tag="junk2")
                cn = spool.tile([K, 1], f32)
                nc.scalar.activation(junkC[:], C_f[:], AF.Square,
                                     accum_out=cn[:])
                G_ps = psM.tile([K, K], f32, tag="m")
                for fc in range(FCH):
                    ct = CT[:, fc * K:(fc + 1) * K]
                    nc.tensor.matmul(G_ps[:], ct, ct,
                                     start=(fc == 0), stop=(fc == FCH - 1))
                Grs = spool.tile([K, 1], f32)
                nc.vector.tensor_reduce(Grs[:], G_ps[:], AX.X, ALU.add)
                ones64 = spool.tile([K, 1], f32)
                nc.vector.memset(ones64[:], 1.0)
                ones1 = spool.tile([1, K], f32)
                nc.vector.memset(ones1[:], 1.0)
                S_ps = psM.tile([1, 1], f32, tag="m")
                nc.tensor.matmul(S_ps[:], cn[:], ones64[:], start=True, stop=True)
                S_sb = spool.tile([1, 1], f32)
                nc.vector.tensor_copy(S_sb[:], S_ps[:])
                Sbc_ps = psM.tile([K, 1], f32, tag="m")
                nc.tensor.matmul(Sbc_ps[:], ones1[:], S_sb[:], start=True,
                                 stop=True)

                d1 = spool.tile([K, 1], f32)
                nc.vector.tensor_scalar(d1[:], cn[:], float(K), ALPHA,
                                        ALU.mult, ALU.add)
                d2 = spool.tile([K, 1], f32)
                nc.vector.scalar_tensor_tensor(d2[:], Grs[:], -2.0, d1[:],
                                               ALU.mult, ALU.add)
                denom = spool.tile([K, 1], f32)
                nc.vector.tensor_tensor(denom[:], d2[:], Sbc_ps[:], ALU.add)
                invd = spool.tile([K, 1], f32)
                nc.vector.reciprocal(invd[:], denom[:])
                u = spool.tile([K, 1], f32)      # cn/denom
                nc.vector.tensor_tensor(u[:], cn[:], invd[:], ALU.mult)
                bcoef = spool.tile([K, 1], f32)  # GAMMA*cw/denom
                nc.vector.tensor_tensor(bcoef[:], cw_t[:], invd[:], ALU.mult)
                # per-row gathers: ivr_all[:, t] = invd[i(n)], [:, T+t] = u[i(n)]
                iv_u = spool.tile([K, 2], f32)   # [invd | u]
                nc.vector.tensor_copy(iv_u[:, 0:1], invd[:])
                nc.vector.tensor_copy(iv_u[:, 1:2], u[:])
                iv_ps = psM.tile([128, 2 * T], f32, tag="m")
                for t in range(T):
                    nc.tensor.matmul(iv_ps[:, 2 * t:2 * t + 2],
                                     LT_f[:, t * 128:(t + 1) * 128], iv_u[:],
                                     start=True, stop=True)
                ivr_all = cpool.tile([128, 2 * T], f32)
                nc.scalar.copy(ivr_all[:], iv_ps[:])
                count = spool.tile([K, 1], f32)
                nc.vector.tensor_reduce(count[:], LT_f[:], AX.X, ALU.add)

            # ---------------- main loop over row tiles ----------------
            Sx_ps = psA.tile([K, F], f32)  # 4 banks, persistent accumulator
            loss_strip = cpool.tile([128, T], f32, tag="strip")
            Sxs = cpool.tile([K, F], f32, tag="sxs")
            bounce_in = dpool.tile([K, F + 1], f32)
            bounce_out = dpool.tile([K, F + 1], f32, addr_space="Shared")
            rg = [list(range(N_CORES))]
            NG = T // GRP
            with tc.tile_pool(name="psCI", bufs=2, space="PSUM") as psCI:
                for g in range(NG):
                    xsq_g = spool.tile([128, GRP], f32, tag=f"xsq{g % 2}")
                    th0_g = spool.tile([128, GRP], f32, tag=f"th0{g % 2}")
                    th1_g = spool.tile([128, GRP], f32, tag=f"th1{g % 2}")
                    for i in range(GRP):
                        t = g * GRP + i
                        X_t = xpool.tile([128, F], f32r, tag="x")
                        nc.sync.dma_start(X_t[:], Xv[:, t, :])
                        # Sx first: frees the AllReduce as soon as X_15 lands
                        for j in range(4):
                            nc.tensor.matmul(Sx_ps[:, j * 512:(j + 1) * 512],
                                             L_r[:, t * K:(t + 1) * K],
                                             X_t[:, j * 512:(j + 1) * 512],
                                             start=(t == 0), stop=(t == T - 1))
                        if t == T - 1:
                            # scaled partial out + AllReduce, overlapping the
                            # rest of the loss pipeline
                            nc.scalar.mul(Sxs[:], Sx_ps[:], bcoef[:])
                            nc.sync.dma_start(bounce_in[:, 0:F], Sxs[:])
                            nc.sync.dma_start(bounce_in[:, F:F + 1], count[:])
                            nc.gpsimd.collective_compute(
                                "AllReduce", ALU.add, replica_groups=rg,
                                ins=[bounce_in.opt()], outs=[bounce_out.opt()])
                        lt16 = LT16[:, t * 128:(t + 1) * 128]
                        for h in range(2):
                            ci_ps = psCI.tile([128, F // 2], f32, tag="ci")
                            for j in range(2):
                                cs = slice(h * 1024 + j * 512,
                                           h * 1024 + (j + 1) * 512)
                                nc.tensor.matmul(ci_ps[:, j * 512:(j + 1) * 512],
                                                 lt16, C16[:, cs],
                                                 start=True, stop=True)
                            junk1 = jpool.tile([128, F // 2], f32, tag="junk1")
                            # fused x*ci row-dot (tensor_tensor_reduce is
                            # broken on this runtime; scalar_tensor_tensor
                            # with accum_out is the working equivalent)
                            nc.vector.scalar_tensor_tensor(
                                junk1[:],
                                X_t[:, h * 1024:(h + 1) * 1024].bitcast(f32),
                                1.0, ci_ps[:],
                                ALU.mult, ALU.mult,
                                accum_out=(th0_g[:, i:i + 1] if h == 0
                                           else th1_g[:, i:i + 1]))
                        junk2 = jpool.tile([128, F], f32, tag="junk2")
                        nc.scalar.activation(junk2[:], X_t[:].bitcast(f32),
                                             AF.Square,
                                             accum_out=xsq_g[:, i:i + 1])
                    # batched per-row fixup for the group: loss =
                    # (xsq - 2*(th0+th1))*invd_row + u_row
                    trow_g = spool.tile([128, GRP], f32, tag=f"trow{g % 2}")
                    nc.vector.tensor_tensor(trow_g[:], th0_g[:], th1_g[:],
                                            ALU.add)
                    tmp_g = spool.tile([128, GRP], f32, tag=f"tmp{g % 2}")
                    nc.vector.scalar_tensor_tensor(tmp_g[:], trow_g[:], -2.0,
                                                   xsq_g[:], ALU.mult, ALU.add)
                    nc.vector.tensor_tensor(
                        tmp_g[:], tmp_g[:],
                        ivr_all[:, 2 * g * GRP:2 * (g + 1) * GRP:2], ALU.mult)
                    nc.vector.tensor_tensor(
                        loss_strip[:, g * GRP:(g + 1) * GRP], tmp_g[:],
                        ivr_all[:, 2 * g * GRP + 1:2 * (g + 1) * GRP:2],
                        ALU.add)
                nc.sync.dma_start(
                    loss_out.rearrange("(p t) o -> p t o", t=T),
                    loss_strip[:].rearrange("p (t o) -> p t o", o=1))

            # ---------------- cluster update ----------------
            red = cpool.tile([K, F + 1], f32, tag="red")
            nc.sync.dma_start(red[:], bounce_out[:])
            q = spool.tile([K, 1], f32)
            nc.vector.tensor_scalar(q[:], red[:, F:F + 1], bcoef[:], 1.0,
                                    ALU.mult, ALU.add)
            newC = cpool.tile([K, F], f32, tag="newC")
            nc.vector.scalar_tensor_tensor(newC[:], C_f[:], q[:],
                                           red[:, 0:F], ALU.mult,
                                           ALU.subtract)
            nc.sync.dma_start(clus_out[:], newC[:])

    nc.compile()
    return nc


def _get_nc():
    global _CACHED_NC
    if _CACHED_NC is None:
        _CACHED_NC = _build()
    return _CACHED_NC


def _in_maps(features, labels, cluster, class_weight):
    features = np.ascontiguousarray(features, dtype=np.float32)
    labels = np.ascontiguousarray(labels, dtype=np.float32)
    cluster = np.ascontiguousarray(cluster, dtype=np.float32)
    cw = np.ascontiguousarray(class_weight, dtype=np.float32).reshape(K, 1)
    eye = np.eye(128, dtype=np.float32)
    maps = []
    for c in range(N_CORES):
        sl = slice(c * NLOC, (c + 1) * NLOC)
        maps.append({
            "features_r": np.ascontiguousarray(features[sl]),
            "labels_f": np.ascontiguousarray(labels[sl]),
            "cluster_f": cluster,
            "class_weight": cw,
            "identity": eye,
        })
    return maps


def run(features, labels, cluster, class_weight, **run_kwargs):
    nc = _get_nc()
    maps = _in_maps(features, labels, cluster, class_weight)
    res = bass_utils.run_bass_kernel_spmd(
        nc, maps, core_ids=list(range(N_CORES)), **run_kwargs)
    loss = np.concatenate(
        [res.results[c]["loss_out"] for c in range(N_CORES)], axis=0)
    new_cluster = res.results[0]["cluster_out"]
    return (loss, new_cluster), res


def kernel(features, labels, cluster, class_weight):
    (loss, new_cluster), _ = run(features, labels, cluster, class_weight)
    return loss, new_cluster


# revision 13
# speedup vs baseline: 1.1073x; 1.0447x over previous
"""ClusterLayer (vq_codebook) Trainium2 kernel.

Math (labels are one-hot over K clusters, i(n) = argmax labels[n]):
  nume[n]  = ||x_n - c_{i(n)}||^2 = ||x_n||^2 - 2 x_n.c_{i(n)} + ||c_{i(n)}||^2
  denom[k] = sum_j ||c_k - c_j||^2 + ALPHA = K*cn[k] + S - 2*Grs[k] + ALPHA
  loss[n]  = nume[n] / denom[i(n)]
  new_cluster[k] = cluster[k]*(1 + b[k]*count[k]) - b[k]*Sx[k]   with
  b[k] = GAMMA*cw[k]/denom[k], Sx = L^T @ X (segment sums), count = colsum(L)

Sharding: data-parallel over N across 8 cores. Rows are assigned to
row-tiles by n % 16 (tile j holds rows {p*16+j}) so the labels load is one
contiguous DMA; every per-row quantity is permutation-invariant.

Schedule: per tile the Sx matmuls are emitted before the ci/loss work, so
the Sx accumulator finishes as soon as the last X tile lands; the single
AllReduce of [Sx | count] launches right then, overlapping the remaining
per-row loss work, and only its latency tail is exposed.

Engine split: PE does Sx (fp32r) + ci = L@C (bf16; one-hot labels exact,
cluster rounding washes out in nume ~ 2048); DVE does the fp32 x.ci row
dots (fp32 tensor_tensor is 1x on DVE - hard floor); ACT does ||x||^2
squares and all PSUM->SBUF copies; per-row scalar fixups are batched in
groups of 4 row tiles.
"""

import sys

if "/opt/trn_rl_repo" not in sys.path:
    sys.path.insert(0, "/opt/trn_rl_repo")

import numpy as np

import concourse.bass as bass
import concourse.bacc as bacc
import concourse.tile as tile
import concourse.mybir as mybir
from concourse import bass_utils

N, K, F = 16384, 64, 2048
ALPHA, GAMMA = 1.0, 1.0
N_CORES = 8
NLOC = N // N_CORES          # 2048 rows per core
T = NLOC // 128              # 16 row tiles per core
FCH = F // 128               # 16 column chunks of cluster
GRP = 4                      # row tiles per small-op batch
f32 = mybir.dt.float32
f32r = mybir.dt.float32r
bf16 = mybir.dt.bfloat16
AF = mybir.ActivationFunctionType
ALU = mybir.AluOpType
AX = mybir.AxisListType

_CACHED_NC = None


def _build():
    nc = bacc.Bacc("TRN2", target_bir_lowering=False, debug=False,
                   num_devices=N_CORES)
    Xr = nc.dram_tensor("features_r", [NLOC, F], f32r, kind="ExternalInput").ap()
    Lf = nc.dram_tensor("labels_f", [NLOC, K], f32, kind="ExternalInput").ap()
    Cf = nc.dram_tensor("cluster_f", [K, F], f32, kind="ExternalInput").ap()
    cw = nc.dram_tensor("class_weight", [K, 1], f32, kind="ExternalInput").ap()
    ident = nc.dram_tensor("identity", [128, 128], f32, kind="ExternalInput").ap()
    loss_out = nc.dram_tensor("loss_out", [NLOC, 1], f32, kind="ExternalOutput").ap()
    clus_out = nc.dram_tensor("cluster_out", [K, F], f32, kind="ExternalOutput").ap()

    # row-tile view: tile j holds rows {p*16 + j, p in 0..127}
    Xv = Xr.rearrange("(p t) f -> p t f", t=T)
    Lv = Lf.rearrange("(p t) k -> p t k", t=T)

    with tile.TileContext(nc) as tc:
        with tc.tile_pool(name="const", bufs=1) as cpool, \
             tc.tile_pool(name="xin", bufs=6) as xpool, \
             tc.tile_pool(name="junk", bufs=1) as jpool, \
             tc.tile_pool(name="small", bufs=1) as spool, \
             tc.tile_pool(name="psSx", bufs=1, space="PSUM") as psA, \
             tc.tile_pool(name="dram", bufs=1, space="DRAM") as dpool:

            # ---------------- constant loads ----------------
            C_f = cpool.tile([K, F], f32)
            nc.sync.dma_start(C_f[:], Cf[:])
            id_t = cpool.tile([128, 128], f32)
            nc.sync.dma_start(id_t[:], ident[:])
            L_f = cpool.tile([128, T * K], f32)  # tile j at cols [j*K,(j+1)*K)
            nc.sync.dma_start(L_f[:].rearrange("p (t k) -> p t k", t=T), Lv)
            L_r = cpool.tile([128, T * K], f32r)
            nc.sync.dma_start(L_r[:], L_f[:].bitcast(f32r))  # rebless f32->f32r
            C16 = cpool.tile([K, F], bf16)
            nc.scalar.copy(C16[:], C_f[:])
            cw_t = spool.tile([K, 1], f32)
            nc.sync.dma_start(cw_t[:], cw[:])

            # ---------------- transposes via PE ----------------
            LT_f = cpool.tile([K, NLOC], f32)    # labels^T (f32)
            LT16 = cpool.tile([K, NLOC], bf16)   # labels^T (bf16, ci matmul)
            CT = cpool.tile([128, FCH * K], f32)  # cluster^T chunks [128_f, K]
            with tc.tile_pool(name="psT", bufs=2, space="PSUM") as psT:
                for t in range(T):
                    sl = slice(t * 128, (t + 1) * 128)
                    ps = psT.tile([K, 128], f32, tag="tr")
                    nc.tensor.transpose(ps[:], L_f[:, t * K:(t + 1) * K], id_t[:])
                    nc.scalar.copy(LT_f[:, sl], ps[:])
                    nc.scalar.copy(LT16[:, sl], ps[:])
                for fc in range(FCH):
                    ps2 = psT.tile([128, K], f32, tag="tr2")
                    nc.tensor.transpose(ps2[:], C_f[:, fc * 128:(fc + 1) * 128],
                                        id_t[0:K, 0:K])
                    nc.scalar.copy(CT[:, fc * K:(fc + 1) * K], ps2[:])

            # ---------------- cluster geometry (prelude) ----------------
            with tc.tile_pool(name="psM", bufs=1, space="PSUM") as psM:
                junkC = jpool.tile([K, F], f32, tag="junk2")
                cn = spool.tile([K, 1], f32)
                nc.scalar.activation(junkC[:], C_f[:], AF.Square,
                                     accum_out=cn[:])
                G_ps = psM.tile([K, K], f32, tag="m")
                for fc in range(FCH):
                    ct = CT[:, fc * K:(fc + 1) * K]
                    nc.tensor.matmul(G_ps[:], ct, ct,
                                     start=(fc == 0), stop=(fc == FCH - 1))
                Grs = spool.tile([K, 1], f32)
                nc.vector.tensor_reduce(Grs[:], G_ps[:], AX.X, ALU.add)
                ones64 = spool.tile([K, 1], f32)
                nc.vector.memset(ones64[:], 1.0)
                ones1 = spool.tile([1, K], f32)
                nc.vector.memset(ones1[:], 1.0)
                S_ps = psM.tile([1, 1], f32, tag="m")
                nc.tensor.matmul(S_ps[:], cn[:], ones64[:], start=True, stop=True)
                S_sb = spool.tile([1, 1], f32)
                nc.vector.tensor_copy(S_sb[:], S_ps[:])
                Sbc_ps = psM.tile([K, 1], f32, tag="m")
                nc.tensor.matmul(Sbc_ps[:], ones1[:], S_sb[:], start=True,
                                 stop=True)

                d1 = spool.tile([K, 1], f32)
                nc.vector.tensor_scalar(d1[:], cn[:], float(K), ALPHA,
                                        ALU.mult, ALU.add)
                d2 = spool.tile([K, 1], f32)
                nc.vector.scalar_tensor_tensor(d2[:], Grs[:], -2.0, d1[:],
                                               ALU.mult, ALU.add)
                denom = spool.tile([K, 1], f32)
                nc.vector.tensor_tensor(denom[:], d2[:], Sbc_ps[:], ALU.add)
                invd = spool.tile([K, 1], f32)
                nc.vector.reciprocal(invd[:], denom[:])
                u = spool.tile([K, 1], f32)      # cn/denom
                nc.vector.tensor_tensor(u[:], cn[:], invd[:], ALU.mult)
                bcoef = spool.tile([K, 1], f32)  # GAMMA*cw/denom
                nc.vector.tensor_tensor(bcoef[:], cw_t[:], invd[:], ALU.mult)
                # per-row gathers: ivr_all[:, t] = invd[i(n)], [:, T+t] = u[i(n)]
                iv_u = spool.tile([K, 2], f32)   # [invd | u]
                nc.vector.tensor_copy(iv_u[:, 0:1], invd[:])
                nc.vector.tensor_copy(iv_u[:, 1:2], u[:])
                iv_ps = psM.tile([128, 2 * T], f32, tag="m")
                for t in range(T):
                    nc.tensor.matmul(iv_ps[:, 2 * t:2 * t + 2],
                                     LT_f[:, t * 128:(t + 1) * 128], iv_u[:],
                                     start=True, stop=True)
                ivr_all = cpool.tile([128, 2 * T], f32)
                nc.scalar.copy(ivr_all[:], iv_ps[:])
                count = spool.tile([K, 1], f32)
                nc.vector.tensor_reduce(count[:], LT_f[:], AX.X, ALU.add)

            # ---------------- main loop over row tiles ----------------
            Sx_ps = psA.tile([K, F], f32)  # 4 banks, persistent accumulator
            loss_strip = cpool.tile([128, T], f32, tag="strip")
            Sxs = cpool.tile([K, F], f32, tag="sxs")
            bounce_in = dpool.tile([K, F + 1], f32)
            bounce_out = dpool.tile([K, F + 1], f32, addr_space="Shared")
            rg = [list(range(N_CORES))]
            NG = T // GRP
            with tc.tile_pool(name="psCI", bufs=4, space="PSUM") as psCI:
                for g in range(NG):
                    xsq_g = spool.tile([128, GRP], f32, tag=f"xsq{g % 2}")
                    th_a = spool.tile([128, GRP], f32, tag=f"th0{g % 2}")
                    th_b = spool.tile([128, GRP], f32, tag=f"th1{g % 2}")
                    th_c = spool.tile([128, GRP], f32, tag=f"th2{g % 2}")
                    th_d = spool.tile([128, GRP], f32, tag=f"th3{g % 2}")
                    th_g = [th_a, th_b, th_c, th_d]
                    for i in range(GRP):
                        t = g * GRP + i
                        X_t = xpool.tile([128, F], f32r, tag="x")
                        nc.sync.dma_start(X_t[:], Xv[:, t, :])
                        # Sx first: frees the AllReduce as soon as X_15 lands
                        for j in range(4):
                            nc.tensor.matmul(Sx_ps[:, j * 512:(j + 1) * 512],
                                             L_r[:, t * K:(t + 1) * K],
                                             X_t[:, j * 512:(j + 1) * 512],
                                             start=(t == 0), stop=(t == T - 1))
                        if t == T - 1:
                            # scaled partial out + AllReduce, overlapping the
                            # rest of the loss pipeline
                            nc.scalar.mul(Sxs[:], Sx_ps[:], bcoef[:])
                            nc.sync.dma_start(bounce_in[:, 0:F], Sxs[:])
                            nc.sync.dma_start(bounce_in[:, F:F + 1], count[:])
                            nc.gpsimd.collective_compute(
                                "AllReduce", ALU.add, replica_groups=rg,
                                ins=[bounce_in.opt()], outs=[bounce_out.opt()])
                        lt16 = LT16[:, t * 128:(t + 1) * 128]
                        for h in range(4):
                            ci_ps = psCI.tile([128, F // 4], f32, tag="ci")
                            cs = slice(h * 512, (h + 1) * 512)
                            nc.tensor.matmul(ci_ps[:], lt16, C16[:, cs],
                                             start=True, stop=True)
                            junk1 = jpool.tile([128, F // 4], f32, tag="junk1")
                            # fused x*ci row-dot (tensor_tensor_reduce is
                            # broken on this runtime; scalar_tensor_tensor
                            # with accum_out is the working equivalent)
                            nc.vector.scalar_tensor_tensor(
                                junk1[:],
                                X_t[:, cs].bitcast(f32),
                                1.0, ci_ps[:],
                                ALU.mult, ALU.mult,
                                accum_out=(th_g[h][:, i:i + 1]))
                        junk2 = jpool.tile([128, F], f32, tag="junk2")
                        nc.scalar.activation(junk2[:], X_t[:].bitcast(f32),
                                             AF.Square,
                                             accum_out=xsq_g[:, i:i + 1])
                    # batched per-row fixup for the group: loss =
                    # (xsq - 2*(th0+th1))*invd_row + u_row
                    trow_g = spool.tile([128, GRP], f32, tag=f"trow{g % 2}")
                    nc.vector.tensor_tensor(trow_g[:], th_g[0][:], th_g[1][:],
                                            ALU.add)
                    nc.vector.tensor_tensor(th_g[2][:], th_g[2][:], th_g[3][:],
                                            ALU.add)
                    nc.vector.tensor_tensor(trow_g[:], trow_g[:], th_g[2][:],
                                            ALU.add)
                    tmp_g = spool.tile([128, GRP], f32, tag=f"tmp{g % 2}")
                    nc.vector.scalar_tensor_tensor(tmp_g[:], trow_g[:], -2.0,
                                                   xsq_g[:], ALU.mult, ALU.add)
                    nc.vector.tensor_tensor(
                        tmp_g[:], tmp_g[:],
                        ivr_all[:, 2 * g * GRP:2 * (g + 1) * GRP:2], ALU.mult)
                    nc.vector.tensor_tensor(
                        loss_strip[:, g * GRP:(g + 1) * GRP], tmp_g[:],
                        ivr_all[:, 2 * g * GRP + 1:2 * (g + 1) * GRP:2],
                        ALU.add)
                nc.sync.dma_start(
                    loss_out.rearrange("(p t) o -> p t o", t=T),
                    loss_strip[:].rearrange("p (t o) -> p t o", o=1))

            # ---------------- cluster update ----------------
            red = cpool.tile([K, F + 1], f32, tag="red")
            nc.sync.dma_start(red[:], bounce_out[:])
            q = spool.tile([K, 1], f32)
            nc.vector.tensor_scalar(q[:], red[:, F:F + 1], bcoef[:], 1.0,
                                    ALU.mult, ALU.add)
            newC = cpool.tile([K, F], f32, tag="newC")
            nc.vector.scalar_tensor_tensor(newC[:], C_f[:], q[:],
                                           red[:, 0:F], ALU.mult,
                                           ALU.subtract)
            nc.sync.dma_start(clus_out[:], newC[:])

    nc.compile()
    return nc


def _get_nc():
    global _CACHED_NC
    if _CACHED_NC is None:
        _CACHED_NC = _build()
    return _CACHED_NC


def _in_maps(features, labels, cluster, class_weight):
    features = np.ascontiguousarray(features, dtype=np.float32)
    labels = np.ascontiguousarray(labels, dtype=np.float32)
    cluster = np.ascontiguousarray(cluster, dtype=np.float32)
    cw = np.ascontiguousarray(class_weight, dtype=np.float32).reshape(K, 1)
    eye = np.eye(128, dtype=np.float32)
    maps = []
    for c in range(N_CORES):
        sl = slice(c * NLOC, (c + 1) * NLOC)
        maps.append({
            "features_r": np.ascontiguousarray(features[sl]),
            "labels_f": np.ascontiguousarray(labels[sl]),
            "cluster_f": cluster,
            "class_weight": cw,
            "identity": eye,
        })
    return maps


def run(features, labels, cluster, class_weight, **run_kwargs):
    nc = _get_nc()
    maps = _in_maps(features, labels, cluster, class_weight)
    res = bass_utils.run_bass_kernel_spmd(
        nc, maps, core_ids=list(range(N_CORES)), **run_kwargs)
    loss = np.concatenate(
        [res.results[c]["loss_out"] for c in range(N_CORES)], axis=0)
    new_cluster = res.results[0]["cluster_out"]
    return (loss, new_cluster), res


def kernel(features, labels, cluster, class_weight):
    (loss, new_cluster), _ = run(features, labels, cluster, class_weight)
    return loss, new_cluster
